# revision 12
# baseline (speedup 1.0000x reference)
"""Trainium2 Bass kernel v9 for nn_AaD_MAPU (retrieval kNN shortlist).

Drain-bound design. The PE computes the full fp8 distance matrix
(512 q x 12544 cols per core) at full clock (~21us); the binding
constraint is PSUM-exit bandwidth: every dot value must leave PSUM
through Act (0.83 ns/v) or DVE (1.04 ns/v); GPSIMD has no PSUM port,
DMA cannot read PSUM. Whole-tile consumers (2048 v per instruction)
amortize init overheads. Two paths, one per tile:

  B  : DVE tensor_reduce from PSUM -> per-128-col bucket maxes (fp32)
  A2 : Act exp(psum/16) -> fp8 tile, DMA ships it to the host.
       exp-companding keeps ~bf16-class ranking precision at the top
       of the dot range in 1 byte, halving ship bandwidth.

PSUM: two 4-bank tile buffers (PE fills one while a consumer drains
the other). Input: whole fbt (fnt + bank shard, fp8) is SBUF-resident,
chunked DMAs issued upfront. The small 256-col tile runs first and the
final bred piece is a tiny DMA so the tail chain is short.

Host: bucket shortlist (128-col bucket maxes from both paths), exact
fp32 re-rank of top bucket members, fp64 loss.
"""

from contextlib import ExitStack

import numpy as np

import concourse.bass as bass
import concourse.tile as tile
from concourse import bacc, mybir
from concourse.bass_utils import run_bass_kernel_spmd

B, D, N, C, K = 512, 512, 100000, 64, 5
EPS = 1e-12
NCORES = 8
NSHARD = 12544
NPAD = NSHARD * NCORES
FSCALE = 16.0

NT = 25                      # tiles: 24 x 512 + 1 x 256
TILE_W = [512] * 24 + [256]
N_WARMUP = 7
N_A2 = 13                    # tiles shipped via Act exp->fp8; rest DVE
TILE_ORDER = [24] + list(range(24))
SLABS = [3, 3, 3, 2, 2]      # A2 tiles per ship DMA
BRED_SPLIT = 16              # bred[:, :16] ships mid-kernel

_F32 = mybir.dt.float32
_FP8 = mybir.dt.float8e4

_cache = {}


def _assign_paths():
    """per processing position -> 'B' | '2'. Position 0 (tile 24, small)
    and the last position forced B; Bresenham-spread otherwise."""
    n = NT
    forced_b = {0, n - 1}
    quota = {"B": n - N_A2 - len(forced_b), "2": N_A2}
    issued = {"B": 0, "2": 0}
    out = []
    for i in range(n):
        if i in forced_b:
            out.append("B")
            continue
        cand = [p for p in ("B", "2") if issued[p] < quota[p]]
        p = min(cand, key=lambda q: (issued[q] + 0.5) / quota[q])
        issued[p] += 1
        out.append(p)
    return out


def _build_module():
    nc = bacc.Bacc("TRN2", target_bir_lowering=False, debug=False,
                   num_devices=NCORES)
    # columns 0:512 = f_norm.T * FSCALE, columns 512: = bank shard.T
    fbt_d = nc.dram_tensor("fbt", [D, B + NSHARD], _FP8,
                           kind="ExternalInput").ap()
    raw_out = nc.dram_tensor("raw_out", [128, max(N_A2, 1), 2, 2, 512], _FP8,
                             kind="ExternalOutput").ap()
    # bred[p, t, h, bi, g]: B-path bucket maxes (fp32, units of 16*d)
    bred_out = nc.dram_tensor("bred_out", [128, NT, 2, 2, 4], _F32,
                              kind="ExternalOutput").ap()

    paths = _assign_paths()

    with tile.TileContext(nc) as tc, ExitStack() as ctx:
        const = ctx.enter_context(tc.tile_pool(name="const", bufs=1))
        dp_pool = ctx.enter_context(tc.tile_pool(name="dp", bufs=2, space="PSUM"))

        # PE warm-up + act-table preload during the initial DMA wait.
        wu_sb = const.tile([128, 512], _F32)
        nc.gpsimd.memset(wu_sb[:], 0.0)
        wu_act = const.tile([128, 1], _F32, name="wu_act")
        nc.scalar.activation(out=wu_act[:], in_=wu_sb[:, 0:1],
                             func=mybir.ActivationFunctionType.Exp)
        wu_ps = dp_pool.tile([128, 2, 2, 512], _F32, tag="dp")
        wu_r = wu_sb[:].bitcast(_FP8).rearrange("p (c j) -> p c j", c=4)
        for _ in range(N_WARMUP):
            nc.tensor.matmul(wu_ps[:, 0, 0], lhsT=wu_r[:, 0:2, :128],
                             rhs=wu_r[:, 0:2],
                             start=True, stop=True,
                             perf_mode=mybir.MatmulPerfMode.DoubleRow)
        wu_reuse = [wu_ps]

        # SBUF-resident fbt: ch0 = fnt + tile24 + tile0, then tile pairs.
        # All input DMAs issued upfront, no deps.
        chw = [512 + 256 + 512] + [1024] * 11 + [512]
        chunks = []
        j0 = 0
        for k, w in enumerate(chw):
            ch = const.tile([128, 4, w], _FP8, name=f"ch{k}")
            nc.sync.dma_start(
                ch[:], fbt_d[:, j0:j0 + w].rearrange("(c p) j -> p c j", p=128))
            chunks.append(ch)
            j0 += w

        fnt_sb = chunks[0][:, :, 0:512]

        # fbt column ranges: host lays tile 24 at fbt cols [512, 768),
        # tile t (t<24) at [768 + 512*t, 768 + 512*t + 512)
        def tile_rhs(t):
            if t == 24:
                return chunks[0][:, :, 512:768]
            j = 768 + 512 * t
            if t == 0:
                return chunks[0][:, :, 768:1280]
            k = (t + 1) // 2
            off = j - sum(chw[:k])
            return chunks[k][:, :, off:off + 512]

        bredA = const.tile([128, BRED_SPLIT, 2, 2, 4], _F32, name="bredA")
        bredB = const.tile([128, NT - BRED_SPLIT, 2, 2, 4], _F32, name="bredB")
        tmps = [const.tile([128, w, 2, 2, 512], _FP8, name=f"tmp{i}")
                for i, w in enumerate(SLABS)]
        slab_edges = np.cumsum(SLABS).tolist()

        ti = 0   # A2 tile slot
        si = 0   # slab index
        s0 = 0   # slab start slot
        bred_shipped = False
        for pos, t in enumerate(TILE_ORDER):
            W = TILE_W[t]
            fbt = tile_rhs(t)
            p = paths[pos]
            if wu_reuse:
                dp = wu_reuse.pop()
            else:
                dp = dp_pool.tile([128, 2, 2, 512], _F32, tag="dp")
            for h in range(2):
                for bi in range(2):
                    bc = 2 * h + bi
                    for dc in range(2):
                        nc.tensor.matmul(
                            dp[:, h, bi, :W],
                            lhsT=fnt_sb[:, 2 * dc:2 * dc + 2,
                                        bc * 128:(bc + 1) * 128],
                            rhs=fbt[:, 2 * dc:2 * dc + 2, :W],
                            start=(dc == 0), stop=(dc == 1),
                            perf_mode=mybir.MatmulPerfMode.DoubleRow,
                        )
            if p == "B":
                g = W // 128
                bslot = bredA[:, t] if t < BRED_SPLIT else bredB[:, t - BRED_SPLIT]
                nc.vector.tensor_reduce(
                    out=bslot[:, :, :, :g],
                    in_=dp[:, :, :, :W].rearrange("p a b (g c) -> p a b g c",
                                                  c=128),
                    axis=mybir.AxisListType.X, op=mybir.AluOpType.max)
            else:
                nc.scalar.activation(out=tmps[si][:, ti - s0, :, :, :W],
                                     in_=dp[:, :, :, :W],
                                     func=mybir.ActivationFunctionType.Exp,
                                     scale=1.0 / FSCALE)
                ti += 1
                if ti == slab_edges[si]:
                    nc.sync.dma_start(raw_out[:, s0:ti], tmps[si][:])
                    s0 = ti
                    si += 1
            if pos == BRED_SPLIT + 1 and not bred_shipped:
                # bulk of bred ships mid-pipeline; only a small tail DMA
                # remains after the final reduce
                nc.sync.dma_start(bred_out[:, :BRED_SPLIT], bredA[:])
                bred_shipped = True

        nc.sync.dma_start(bred_out[:, BRED_SPLIT:], bredB[:])

    nc.compile()
    return nc


def _get_module():
    if "nc" not in _cache:
        _cache["nc"] = _build_module()
    return _cache["nc"]


def _host_tables():
    if "tables" in _cache:
        return _cache["tables"]
    paths = _assign_paths()
    fine = []   # (t, src, slot)
    ti = 0
    for pos, t in enumerate(TILE_ORDER):
        if paths[pos] == "B":
            fine.append((t, "B", 0))
        else:
            fine.append((t, "2", ti))
            ti += 1
    _cache["tables"] = (paths, fine)
    return _cache["tables"]


def kernel(features, predictions, fea_bank, score_bank, trg_idx):
    features = np.asarray(features, dtype=np.float32)
    predictions = np.asarray(predictions, dtype=np.float32)
    fea_bank = np.asarray(fea_bank, dtype=np.float32)
    score_bank = np.asarray(score_bank, dtype=np.float32)
    trg_idx = np.asarray(trg_idx, dtype=np.int32)

    sm = predictions - predictions.max(axis=1, keepdims=True)
    np.exp(sm, out=sm)
    sm /= sm.sum(axis=1, keepdims=True)
    nrm = np.maximum(np.sqrt((features * features).sum(axis=1, keepdims=True)),
                     EPS)
    f_norm = features / nrm

    fbp = np.zeros((NPAD, D), dtype=np.float32)
    fbp[:N] = fea_bank
    fbp[trg_idx] = f_norm
    sb = score_bank.copy()
    sb[trg_idx] = sm

    import ml_dtypes
    fp8 = ml_dtypes.float8_e4m3
    fnt_cols = (f_norm.T * FSCALE).astype(np.float32)

    nc = _get_module()
    # device fbt layout: [fnt (512) | tile24 (256) | tiles 0..23 (512 each)]
    in_maps = []
    for c in range(NCORES):
        shard = fbp[c * NSHARD:(c + 1) * NSHARD].T        # [D, NSHARD]
        dev = np.concatenate(
            [fnt_cols, shard[:, 512 * 24:], shard[:, :512 * 24]], axis=1)
        in_maps.append({"fbt": np.ascontiguousarray(dev).astype(fp8)})
    res = run_bass_kernel_spmd(nc, in_maps, core_ids=list(range(NCORES)))

    paths, fine = _host_tables()

    breds = [r["bred_out"] for r in res.results]                   # 16*d fp32
    raws = [r["raw_out"].astype(np.float32) for r in res.results]  # exp(d) fp8

    TOPF = 16   # buckets kept per query

    nf = len(fine)
    # values [128p, 2h, 2bi, NCORES, nf, 4] in d-units
    fv = np.full((128, 2, 2, NCORES, nf, 4), -np.inf, np.float32)
    with np.errstate(divide="ignore"):
        for c in range(NCORES):
            for fi, (t, src, slot) in enumerate(fine):
                g = TILE_W[t] // 128
                if src == "B":
                    fv[:, :, :, c, fi, :g] = breds[c][:, t, :, :, :g] / FSCALE
                else:
                    v = raws[c][:, slot, :, :, :TILE_W[t]]
                    v = v.reshape(128, 2, 2, g, 128).max(axis=4)
                    fv[:, :, :, c, fi, :g] = np.log(v)
    fbase = np.empty((NCORES, nf, 4), np.int64)
    for c in range(NCORES):
        for fi, (t, src, slot) in enumerate(fine):
            for g in range(4):
                fbase[c, fi, g] = c * NSHARD + 512 * t + 128 * g
    fvf = fv.reshape(128, 2, 2, NCORES * nf * 4)
    fbf = fbase.reshape(NCORES * nf * 4)
    selF = np.argpartition(-fvf, TOPF, axis=3)[:, :, :, :TOPF]
    rows_hb = (fbf[selF][..., None] + np.arange(128, dtype=np.int64)
               ).reshape(128, 2, 2, TOPF * 128)

    ncand = TOPF * 128
    rows_all = np.zeros((B, ncand), np.int64)
    for h in range(2):
        for bi in range(2):
            q0 = (2 * h + bi) * 128
            rows_all[q0:q0 + 128] = rows_hb[:, h, bi]

    # ---- exact re-rank -----------------------------------------------------
    dots = np.empty((B, ncand), np.float32)
    CH = 64
    for q0 in range(0, B, CH):
        rr = rows_all[q0:q0 + CH]
        vec = fbp[rr.reshape(-1)].reshape(CH, ncand, D)
        dots[q0:q0 + CH] = np.einsum("qkd,qd->qk", vec,
                                     f_norm[q0:q0 + CH], optimize=True)
    dots = np.where(rows_all < N, dots, np.float32(-np.inf))

    # top-6 rows, ties by lower row id (match jax top_k); buckets are
    # disjoint so no dedupe needed.
    order = np.lexsort((rows_all, -dots), axis=1)[:, :K + 1]
    top_idx = np.take_along_axis(rows_all, order, axis=1)

    idx_near = top_idx[:, 1:K + 1]
    score_near = sb[idx_near].astype(np.float64)
    kl = score_near * (np.log(score_near) - sm[:, None, :].astype(np.float64))
    loss = kl.sum(axis=(1, 2)).mean()

    s64 = sm.astype(np.float64)
    neg_pred = (np.square(s64.sum(axis=0)).sum()
                - np.square(s64).sum()) / B

    return np.float32(loss + neg_pred)


# revision 15
# speedup vs baseline: 1.2047x; 1.2047x over previous
"""Trainium2 Bass kernel v10 for nn_AaD_MAPU (retrieval kNN shortlist).

Drain-bound design. The PE computes the full fp8 distance matrix
(512 q x 12544 cols per core) at full clock (~21us); the binding
constraint is PSUM-exit bandwidth: every dot value must leave PSUM
through Act (0.83 ns/v) or DVE (1.04 ns/v); GPSIMD has no PSUM port,
DMA cannot read PSUM. Two paths per half-tile [128, 2, 512]:

  B  : DVE tensor_reduce from PSUM -> per-128-col bucket maxes (fp32)
  A2 : Act exp(psum/16) -> fp8 tile, DMA ships it to the host.
       exp-companding keeps ~bf16-class ranking precision at the top
       of the dot range in 1 byte, halving ship bandwidth.

Input stream: whole fbt (fnt + bank shard, fp8) is SBUF-resident; 13
chunked DMAs (2 tiles each) issued upfront on SP so the DMA engine
streams back-to-back. Ship DMAs are issued from the DVE and GPSIMD
queues (alternating) so their act-completion waits never serialize the
SP input stream or each other. bred ships in two pieces (bulk
mid-kernel, tiny tail).

Host: bucket shortlist (128-col bucket maxes from both paths), exact
fp32 re-rank of top bucket members, fp64 loss.
"""

from contextlib import ExitStack

import numpy as np

import concourse.bass as bass
import concourse.tile as tile
from concourse import bacc, mybir
from concourse.bass_utils import run_bass_kernel_spmd

B, D, N, C, K = 512, 512, 100000, 64, 5
EPS = 1e-12
NCORES = 8
NSHARD = 12544
NPAD = NSHARD * NCORES
FSCALE = 16.0

NT = 25                      # tiles: 24 x 512 + 1 x 256
TILE_W = [512] * 24 + [256]
N_WARMUP = 7
N_A2 = 26                    # halves shipped via Act exp->fp8; rest DVE
BRED_SPLIT = 16              # bred tiles < 16 ship mid-kernel

_F32 = mybir.dt.float32
_FP8 = mybir.dt.float8e4

_cache = {}


def _assign_paths():
    """50 halves -> 'B' | '2'. Tile 24 halves (small) and the final full
    half forced B; Bresenham-spread otherwise."""
    n = 2 * NT
    forced_b = {2 * 24, 2 * 24 + 1, 2 * 23 + 1}
    quota = {"B": n - N_A2 - len(forced_b), "2": N_A2}
    issued = {"B": 0, "2": 0}
    out = []
    for i in range(n):
        if i in forced_b:
            out.append("B")
            continue
        cand = [p for p in ("B", "2") if issued[p] < quota[p]]
        p = min(cand, key=lambda q: (issued[q] + 0.5) / quota[q])
        issued[p] += 1
        out.append(p)
    return out


def _build_module():
    nc = bacc.Bacc("TRN2", target_bir_lowering=False, debug=False,
                   num_devices=NCORES)
    # columns 0:512 = f_norm.T * FSCALE, columns 512: = bank shard.T
    fbt_d = nc.dram_tensor("fbt", [D, B + NSHARD], _FP8,
                           kind="ExternalInput").ap()
    raw_out = nc.dram_tensor("raw_out", [128, max(N_A2, 1), 2, 512], _FP8,
                             kind="ExternalOutput").ap()
    # bred[p, t, h, bi, g]: B-path bucket maxes (fp32, units of 16*d)
    bred_out = nc.dram_tensor("bred_out", [128, NT, 2, 2, 4], _F32,
                              kind="ExternalOutput").ap()

    paths = _assign_paths()

    with tile.TileContext(nc) as tc, ExitStack() as ctx:
        const = ctx.enter_context(tc.tile_pool(name="const", bufs=1))
        dp_pool = ctx.enter_context(tc.tile_pool(name="dp", bufs=4, space="PSUM"))

        # PE warm-up + Exp act-table preload during the initial DMA wait.
        wu_sb = const.tile([128, 512], _F32)
        nc.gpsimd.memset(wu_sb[:], 0.0)
        wu_act = const.tile([128, 1], _F32, name="wu_act")
        nc.scalar.activation(out=wu_act[:], in_=wu_sb[:, 0:1],
                             func=mybir.ActivationFunctionType.Exp)
        wu_ps = dp_pool.tile([128, 2, 512], _F32, tag="dp")
        wu_r = wu_sb[:].bitcast(_FP8).rearrange("p (c j) -> p c j", c=4)
        for _ in range(N_WARMUP):
            nc.tensor.matmul(wu_ps[:, 0], lhsT=wu_r[:, 0:2, :128], rhs=wu_r[:, 0:2],
                             start=True, stop=True,
                             perf_mode=mybir.MatmulPerfMode.DoubleRow)
        wu_reuse = [wu_ps]

        # SBUF-resident fbt in 13 chunks: ch0 = fnt + tile0 (1024 cols),
        # ch k = tiles 2k-1, 2k. All input DMAs issued upfront on SP.
        chw = [1024] * 12 + [768]
        chunks = []
        j0 = 0
        for k, w in enumerate(chw):
            ch = const.tile([128, 4, w], _FP8, name=f"ch{k}")
            nc.sync.dma_start(
                ch[:], fbt_d[:, j0:j0 + w].rearrange("(c p) j -> p c j", p=128))
            chunks.append(ch)
            j0 += w

        fnt_sb = chunks[0][:, :, 0:512]

        def tile_rhs(t):
            # tile t = bank cols [512t, 512t+512) = fbt cols 512+512t ..
            j = 512 + 512 * t
            if t == 0:
                return chunks[0][:, :, 512:512 + TILE_W[0]]
            k = (t + 1) // 2
            off = j - sum(chw[:k])
            return chunks[k][:, :, off:off + TILE_W[t]]

        bredA = const.tile([128, BRED_SPLIT, 2, 2, 4], _F32, name="bredA")
        bredB = const.tile([128, NT - BRED_SPLIT, 2, 2, 4], _F32, name="bredB")
        tmps = [const.tile([128, 2, 512], _FP8, name=f"tmp{i}")
                for i in range(N_A2)]

        ti = 0   # tmp / raw slot index
        bred_shipped = False
        for t in range(NT):
            W = TILE_W[t]
            fbt = tile_rhs(t)
            for h in range(2):
                p = paths[2 * t + h]
                if wu_reuse:
                    dp = wu_reuse.pop()
                else:
                    dp = dp_pool.tile([128, 2, 512], _F32, tag="dp")
                for bi in range(2):
                    bc = 2 * h + bi
                    for dc in range(2):
                        nc.tensor.matmul(
                            dp[:, bi, :W],
                            lhsT=fnt_sb[:, 2 * dc:2 * dc + 2,
                                        bc * 128:(bc + 1) * 128],
                            rhs=fbt[:, 2 * dc:2 * dc + 2, :W],
                            start=(dc == 0), stop=(dc == 1),
                            perf_mode=mybir.MatmulPerfMode.DoubleRow,
                        )
                if p == "B":
                    g = W // 128
                    bslot = (bredA[:, t, h] if t < BRED_SPLIT
                             else bredB[:, t - BRED_SPLIT, h])
                    nc.vector.tensor_reduce(
                        out=bslot[:, :, :g],
                        in_=dp[:, :, :W].rearrange("p a (g c) -> p a g c", c=128),
                        axis=mybir.AxisListType.X, op=mybir.AluOpType.max)
                else:
                    tm = tmps[ti]
                    nc.scalar.activation(out=tm[:, :, :W], in_=dp[:, :, :W],
                                         func=mybir.ActivationFunctionType.Exp,
                                         scale=1.0 / FSCALE)
                    # ship from the idle Pool queue so act-waits never
                    # serialize the SP input stream
                    nc.gpsimd.dma_start(raw_out[:, ti, :, :W], tm[:, :, :W])
                    ti += 1
            if t == BRED_SPLIT + 1 and not bred_shipped:
                nc.sync.dma_start(bred_out[:, :BRED_SPLIT], bredA[:])
                bred_shipped = True

        nc.sync.dma_start(bred_out[:, BRED_SPLIT:], bredB[:])

    nc.compile()
    return nc


def _get_module():
    if "nc" not in _cache:
        _cache["nc"] = _build_module()
    return _cache["nc"]


def _host_tables():
    if "tables" in _cache:
        return _cache["tables"]
    paths = _assign_paths()
    fine = {0: [], 1: []}   # per h: list of (t, src, slot)
    ti = 0
    for t in range(NT):
        for h in range(2):
            if paths[2 * t + h] == "B":
                fine[h].append((t, "B", 0))
            else:
                fine[h].append((t, "2", ti))
                ti += 1
    _cache["tables"] = (paths, fine)
    return _cache["tables"]


def kernel(features, predictions, fea_bank, score_bank, trg_idx):
    features = np.asarray(features, dtype=np.float32)
    predictions = np.asarray(predictions, dtype=np.float32)
    fea_bank = np.asarray(fea_bank, dtype=np.float32)
    score_bank = np.asarray(score_bank, dtype=np.float32)
    trg_idx = np.asarray(trg_idx, dtype=np.int32)

    sm = predictions - predictions.max(axis=1, keepdims=True)
    np.exp(sm, out=sm)
    sm /= sm.sum(axis=1, keepdims=True)
    nrm = np.maximum(np.sqrt((features * features).sum(axis=1, keepdims=True)),
                     EPS)
    f_norm = features / nrm

    fbp = np.zeros((NPAD, D), dtype=np.float32)
    fbp[:N] = fea_bank
    fbp[trg_idx] = f_norm
    sb = score_bank.copy()
    sb[trg_idx] = sm

    import ml_dtypes
    fp8 = ml_dtypes.float8_e4m3
    fnt_cols = (f_norm.T * FSCALE).astype(np.float32)

    nc = _get_module()
    in_maps = [
        {"fbt": np.ascontiguousarray(np.concatenate(
            [fnt_cols, fbp[c * NSHARD:(c + 1) * NSHARD].T],
            axis=1)).astype(fp8)}
        for c in range(NCORES)
    ]
    res = run_bass_kernel_spmd(nc, in_maps, core_ids=list(range(NCORES)))

    paths, fine = _host_tables()

    breds = [r["bred_out"] for r in res.results]                   # 16*d fp32
    raws = [r["raw_out"].astype(np.float32) for r in res.results]  # exp(d) fp8

    TOPF = 16   # buckets kept per query

    rows_h = [None, None]
    for h in range(2):
        fl = fine[h]
        nf = len(fl)
        fv = np.full((128, 2, NCORES, nf, 4), -np.inf, np.float32)
        with np.errstate(divide="ignore"):
            for c in range(NCORES):
                for fi, (t, src, slot) in enumerate(fl):
                    g = TILE_W[t] // 128
                    if src == "B":
                        fv[:, :, c, fi, :g] = breds[c][:, t, h, :, :g] / FSCALE
                    else:
                        v = raws[c][:, slot, :, :TILE_W[t]]
                        v = v.reshape(128, 2, g, 128).max(axis=3)
                        fv[:, :, c, fi, :g] = np.log(v)
        fbase = np.empty((NCORES, nf, 4), np.int64)
        for c in range(NCORES):
            for fi, (t, src, slot) in enumerate(fl):
                for g in range(4):
                    fbase[c, fi, g] = c * NSHARD + 512 * t + 128 * g
        fvf = fv.reshape(128, 2, NCORES * nf * 4)
        fbf = fbase.reshape(NCORES * nf * 4)
        selF = np.argpartition(-fvf, TOPF, axis=2)[:, :, :TOPF]
        rows_h[h] = (fbf[selF][..., None] + np.arange(128, dtype=np.int64)
                     ).reshape(128, 2, TOPF * 128)

    ncand = TOPF * 128
    rows_all = np.zeros((B, ncand), np.int64)
    for h in range(2):
        for bi in range(2):
            q0 = (2 * h + bi) * 128
            rows_all[q0:q0 + 128] = rows_h[h][:, bi]

    # ---- exact re-rank -----------------------------------------------------
    dots = np.empty((B, ncand), np.float32)
    CH = 64
    for q0 in range(0, B, CH):
        rr = rows_all[q0:q0 + CH]
        vec = fbp[rr.reshape(-1)].reshape(CH, ncand, D)
        dots[q0:q0 + CH] = np.einsum("qkd,qd->qk", vec,
                                     f_norm[q0:q0 + CH], optimize=True)
    dots = np.where(rows_all < N, dots, np.float32(-np.inf))

    # top-6 rows, ties by lower row id (match jax top_k); buckets are
    # disjoint so no dedupe needed.
    order = np.lexsort((rows_all, -dots), axis=1)[:, :K + 1]
    top_idx = np.take_along_axis(rows_all, order, axis=1)

    idx_near = top_idx[:, 1:K + 1]
    score_near = sb[idx_near].astype(np.float64)
    kl = score_near * (np.log(score_near) - sm[:, None, :].astype(np.float64))
    loss = kl.sum(axis=(1, 2)).mean()

    s64 = sm.astype(np.float64)
    neg_pred = (np.square(s64.sum(axis=0)).sum()
                - np.square(s64).sum()) / B

    return np.float32(loss + neg_pred)


# revision 17
# speedup vs baseline: 1.3033x; 1.0818x over previous
"""Trainium2 Bass kernel v10 for nn_AaD_MAPU (retrieval kNN shortlist).

Drain-bound design. The PE computes the full fp8 distance matrix
(512 q x 12544 cols per core) at full clock (~21us); the binding
constraint is PSUM-exit bandwidth: every dot value must leave PSUM
through Act (0.83 ns/v) or DVE (1.04 ns/v); GPSIMD has no PSUM port,
DMA cannot read PSUM. Two paths per half-tile [128, 2, 512]:

  B  : DVE tensor_reduce from PSUM -> per-128-col bucket maxes (fp32)
  A2 : Act exp(psum/16) -> fp8 tile, DMA ships it to the host.
       exp-companding keeps ~bf16-class ranking precision at the top
       of the dot range in 1 byte, halving ship bandwidth.

Input stream: whole fbt (fnt + bank shard, fp8) is SBUF-resident; 13
chunked DMAs (2 tiles each) issued upfront on SP so the DMA engine
streams back-to-back. bred ships in two pieces (bulk mid-kernel, tiny tail).

Host: bucket shortlist (128-col bucket maxes from both paths), exact
fp32 re-rank of top bucket members, fp64 loss.
"""

from contextlib import ExitStack

import numpy as np

import concourse.bass as bass
import concourse.tile as tile
from concourse import bacc, mybir
from concourse.bass_utils import run_bass_kernel_spmd

B, D, N, C, K = 512, 512, 100000, 64, 5
EPS = 1e-12
NCORES = 8
NSHARD = 12544
NPAD = NSHARD * NCORES
FSCALE = 16.0

NT = 25                      # tiles: 24 x 512 + 1 x 256
TILE_W = [512] * 24 + [256]
N_WARMUP = 7
N_A2 = 25                    # halves shipped via Act exp->fp8; rest DVE
BRED_SPLIT = 16              # bred tiles < 16 ship mid-kernel

_F32 = mybir.dt.float32
_FP8 = mybir.dt.float8e4

_cache = {}


def _assign_paths():
    """50 halves -> 'B' | '2'. Tile 24 halves (small) and the final full
    half forced B; Bresenham-spread otherwise."""
    n = 2 * NT
    forced_b = {2 * 24, 2 * 24 + 1, 2 * 23 + 1}
    quota = {"B": n - N_A2 - len(forced_b), "2": N_A2}
    issued = {"B": 0, "2": 0}
    out = []
    for i in range(n):
        if i in forced_b:
            out.append("B")
            continue
        cand = [p for p in ("B", "2") if issued[p] < quota[p]]
        p = min(cand, key=lambda q: (issued[q] + 0.5) / quota[q])
        issued[p] += 1
        out.append(p)
    return out


def _build_module():
    nc = bacc.Bacc("TRN2", target_bir_lowering=False, debug=False,
                   num_devices=NCORES)
    # columns 0:512 = f_norm.T * FSCALE, columns 512: = bank shard.T
    fbt_d = nc.dram_tensor("fbt", [D, B + NSHARD], _FP8,
                           kind="ExternalInput").ap()
    raw_out = nc.dram_tensor("raw_out", [128, max(N_A2, 1), 2, 512], _FP8,
                             kind="ExternalOutput").ap()
    # bred[p, t, h, bi, g]: B-path bucket maxes (fp32, units of 16*d)
    bred_out = nc.dram_tensor("bred_out", [128, NT, 2, 2, 4], _F32,
                              kind="ExternalOutput").ap()

    paths = _assign_paths()

    with tile.TileContext(nc) as tc, ExitStack() as ctx:
        const = ctx.enter_context(tc.tile_pool(name="const", bufs=1))
        dp_pool = ctx.enter_context(tc.tile_pool(name="dp", bufs=4, space="PSUM"))

        # PE warm-up + Exp act-table preload during the initial DMA wait.
        wu_sb = const.tile([128, 512], _F32)
        nc.gpsimd.memset(wu_sb[:], 0.0)
        wu_act = const.tile([128, 1], _F32, name="wu_act")
        nc.scalar.activation(out=wu_act[:], in_=wu_sb[:, 0:1],
                             func=mybir.ActivationFunctionType.Exp)
        wu_ps = dp_pool.tile([128, 2, 512], _F32, tag="dp")
        wu_r = wu_sb[:].bitcast(_FP8).rearrange("p (c j) -> p c j", c=4)
        for _ in range(N_WARMUP):
            nc.tensor.matmul(wu_ps[:, 0], lhsT=wu_r[:, 0:2, :128], rhs=wu_r[:, 0:2],
                             start=True, stop=True,
                             perf_mode=mybir.MatmulPerfMode.DoubleRow)
        wu_reuse = [wu_ps]

        # SBUF-resident fbt in 13 chunks: ch0 = fnt + tile0 (1024 cols),
        # ch k = tiles 2k-1, 2k. All input DMAs issued upfront on SP.
        chw = [1024] * 12 + [768]
        chunks = []
        j0 = 0
        for k, w in enumerate(chw):
            ch = const.tile([128, 4, w], _FP8, name=f"ch{k}")
            nc.sync.dma_start(
                ch[:], fbt_d[:, j0:j0 + w].rearrange("(c p) j -> p c j", p=128))
            chunks.append(ch)
            j0 += w

        fnt_sb = chunks[0][:, :, 0:512]

        def tile_rhs(t):
            # tile t = bank cols [512t, 512t+512) = fbt cols 512+512t ..
            j = 512 + 512 * t
            if t == 0:
                return chunks[0][:, :, 512:512 + TILE_W[0]]
            k = (t + 1) // 2
            off = j - sum(chw[:k])
            return chunks[k][:, :, off:off + TILE_W[t]]

        bredA = const.tile([128, BRED_SPLIT, 2, 2, 4], _F32, name="bredA")
        bredB = const.tile([128, NT - BRED_SPLIT, 2, 2, 4], _F32, name="bredB")
        tmps = [const.tile([128, 2, 512], _FP8, name=f"tmp{i}")
                for i in range(N_A2)]

        ti = 0   # tmp / raw slot index
        bred_shipped = False
        for t in range(NT):
            W = TILE_W[t]
            fbt = tile_rhs(t)
            for h in range(2):
                p = paths[2 * t + h]
                if wu_reuse:
                    dp = wu_reuse.pop()
                else:
                    dp = dp_pool.tile([128, 2, 512], _F32, tag="dp")
                for bi in range(2):
                    bc = 2 * h + bi
                    for dc in range(2):
                        nc.tensor.matmul(
                            dp[:, bi, :W],
                            lhsT=fnt_sb[:, 2 * dc:2 * dc + 2,
                                        bc * 128:(bc + 1) * 128],
                            rhs=fbt[:, 2 * dc:2 * dc + 2, :W],
                            start=(dc == 0), stop=(dc == 1),
                            perf_mode=mybir.MatmulPerfMode.DoubleRow,
                        )
                if p == "B":
                    g = W // 128
                    bslot = (bredA[:, t, h] if t < BRED_SPLIT
                             else bredB[:, t - BRED_SPLIT, h])
                    nc.vector.tensor_reduce(
                        out=bslot[:, :, :g],
                        in_=dp[:, :, :W].rearrange("p a (g c) -> p a g c", c=128),
                        axis=mybir.AxisListType.X, op=mybir.AluOpType.max)
                else:
                    tm = tmps[ti]
                    nc.scalar.activation(out=tm[:, :, :W], in_=dp[:, :, :W],
                                         func=mybir.ActivationFunctionType.Exp,
                                         scale=1.0 / FSCALE)
                    nc.sync.dma_start(raw_out[:, ti, :, :W], tm[:, :, :W])
                    ti += 1
            if t == BRED_SPLIT + 1 and not bred_shipped:
                nc.sync.dma_start(bred_out[:, :BRED_SPLIT], bredA[:])
                bred_shipped = True

        nc.sync.dma_start(bred_out[:, BRED_SPLIT:], bredB[:])

    nc.compile()
    return nc


def _get_module():
    if "nc" not in _cache:
        _cache["nc"] = _build_module()
    return _cache["nc"]


def _host_tables():
    if "tables" in _cache:
        return _cache["tables"]
    paths = _assign_paths()
    fine = {0: [], 1: []}   # per h: list of (t, src, slot)
    ti = 0
    for t in range(NT):
        for h in range(2):
            if paths[2 * t + h] == "B":
                fine[h].append((t, "B", 0))
            else:
                fine[h].append((t, "2", ti))
                ti += 1
    _cache["tables"] = (paths, fine)
    return _cache["tables"]


def kernel(features, predictions, fea_bank, score_bank, trg_idx):
    features = np.asarray(features, dtype=np.float32)
    predictions = np.asarray(predictions, dtype=np.float32)
    fea_bank = np.asarray(fea_bank, dtype=np.float32)
    score_bank = np.asarray(score_bank, dtype=np.float32)
    trg_idx = np.asarray(trg_idx, dtype=np.int32)

    sm = predictions - predictions.max(axis=1, keepdims=True)
    np.exp(sm, out=sm)
    sm /= sm.sum(axis=1, keepdims=True)
    nrm = np.maximum(np.sqrt((features * features).sum(axis=1, keepdims=True)),
                     EPS)
    f_norm = features / nrm

    fbp = np.zeros((NPAD, D), dtype=np.float32)
    fbp[:N] = fea_bank
    fbp[trg_idx] = f_norm
    sb = score_bank.copy()
    sb[trg_idx] = sm

    import ml_dtypes
    fp8 = ml_dtypes.float8_e4m3
    fnt_cols = (f_norm.T * FSCALE).astype(np.float32)

    nc = _get_module()
    in_maps = [
        {"fbt": np.ascontiguousarray(np.concatenate(
            [fnt_cols, fbp[c * NSHARD:(c + 1) * NSHARD].T],
            axis=1)).astype(fp8)}
        for c in range(NCORES)
    ]
    res = run_bass_kernel_spmd(nc, in_maps, core_ids=list(range(NCORES)))

    paths, fine = _host_tables()

    breds = [r["bred_out"] for r in res.results]                   # 16*d fp32
    raws = [r["raw_out"].astype(np.float32) for r in res.results]  # exp(d) fp8

    TOPF = 16   # buckets kept per query

    rows_h = [None, None]
    for h in range(2):
        fl = fine[h]
        nf = len(fl)
        fv = np.full((128, 2, NCORES, nf, 4), -np.inf, np.float32)
        with np.errstate(divide="ignore"):
            for c in range(NCORES):
                for fi, (t, src, slot) in enumerate(fl):
                    g = TILE_W[t] // 128
                    if src == "B":
                        fv[:, :, c, fi, :g] = breds[c][:, t, h, :, :g] / FSCALE
                    else:
                        v = raws[c][:, slot, :, :TILE_W[t]]
                        v = v.reshape(128, 2, g, 128).max(axis=3)
                        fv[:, :, c, fi, :g] = np.log(v)
        fbase = np.empty((NCORES, nf, 4), np.int64)
        for c in range(NCORES):
            for fi, (t, src, slot) in enumerate(fl):
                for g in range(4):
                    fbase[c, fi, g] = c * NSHARD + 512 * t + 128 * g
        fvf = fv.reshape(128, 2, NCORES * nf * 4)
        fbf = fbase.reshape(NCORES * nf * 4)
        selF = np.argpartition(-fvf, TOPF, axis=2)[:, :, :TOPF]
        rows_h[h] = (fbf[selF][..., None] + np.arange(128, dtype=np.int64)
                     ).reshape(128, 2, TOPF * 128)

    ncand = TOPF * 128
    rows_all = np.zeros((B, ncand), np.int64)
    for h in range(2):
        for bi in range(2):
            q0 = (2 * h + bi) * 128
            rows_all[q0:q0 + 128] = rows_h[h][:, bi]

    # ---- exact re-rank -----------------------------------------------------
    dots = np.empty((B, ncand), np.float32)
    CH = 64
    for q0 in range(0, B, CH):
        rr = rows_all[q0:q0 + CH]
        vec = fbp[rr.reshape(-1)].reshape(CH, ncand, D)
        dots[q0:q0 + CH] = np.einsum("qkd,qd->qk", vec,
                                     f_norm[q0:q0 + CH], optimize=True)
    dots = np.where(rows_all < N, dots, np.float32(-np.inf))

    # top-6 rows, ties by lower row id (match jax top_k); buckets are
    # disjoint so no dedupe needed.
    order = np.lexsort((rows_all, -dots), axis=1)[:, :K + 1]
    top_idx = np.take_along_axis(rows_all, order, axis=1)

    idx_near = top_idx[:, 1:K + 1]
    score_near = sb[idx_near].astype(np.float64)
    kl = score_near * (np.log(score_near) - sm[:, None, :].astype(np.float64))
    loss = kl.sum(axis=(1, 2)).mean()

    s64 = sm.astype(np.float64)
    neg_pred = (np.square(s64.sum(axis=0)).sum()
                - np.square(s64).sum()) / B

    return np.float32(loss + neg_pred)
